# revision 11
# baseline (speedup 1.0000x reference)
"""DiffAttention Trainium2 kernel.

Full inputs in, full output out. Sharding: 8 cores = (batch b in {0,1}) x
(head-pair p in {0..3}); each core handles one batch element and 2 of the 8
heads (= 4 of the 16 q/k half-heads, 2 v heads, 256 of the 1024 o columns).
Out-projection is column-split: each core produces a full (S, D) partial of
x @ Wo.T restricted to its o columns; host sums the 4 partials per batch.

All device matmuls run in fp16 (1 cycle/row on PE, fp32 PSUM accumulation).
Host pre-transposes operands so every matmul operand is loaded with natural
(contiguous) DMA:
  xT  = x[b].T                  (D, S)   rhs / lhsT for projections
  wqT = (Wq[rows].T) * hd^-0.5  (D, 256) lhsT for q^T projection (scaling folded)
  wkT = Wk[rows].T              (D, 256)
  wvT = Wv[rows].T              (D, 256) rhs for v projection
  woT = Wo[:, cols].T           (256, D) lhsT for out^T projection
Device returns outT_partial (D, S) fp32; host sums 4 partials per batch and
transposes back.

Attention math per head h (half-heads e0=2h, e1=2h+1), per q row:
  u_i = exp(s_i) @ v   (unnormalized), sum_i = exp(s_i) @ 1  (fused: rhs=[v|1])
  o   = u0/sum0 - lam * u1/sum1
  o   = o * rsqrt(mean(o^2)+eps) * (1-lam_init);   out = o @ Wo.T
Scores are computed transposed (keys on partitions, q on free dim) so the
exp'd tiles feed the PV matmul directly as the stationary operand. rsqrt is
computed as exp(-0.5*ln(x) + ln(1-lam_init)) to stay in one ACT table set.
"""

import math

import numpy as np

B = 2
S = 2048
D = 1024
H = 8
HD = 64  # half-head dim
LAMBDA_INIT = 0.8 - 0.6 * math.exp(-0.3 * 6)
EPS = 1e-5

N_CORES = 8
KT = D // 128      # 8 contraction tiles for projections
ST = S // 128      # 16 sequence tiles
NSTRIP = S // 512  # 4 q strips


def _build_program(lam: float, dbg: bool = False):
    import concourse.bass as bass
    import concourse.tile as tile
    from concourse import bacc, mybir
    from concourse.masks import make_identity

    f16 = mybir.dt.float16
    f32 = mybir.dt.float32
    AF = mybir.ActivationFunctionType
    OP = mybir.AluOpType

    nc = bacc.Bacc("TRN2", target_bir_lowering=False, debug=False,
                   num_devices=N_CORES)

    xT = nc.dram_tensor("xT", (D, S), f16, kind="ExternalInput").ap()
    wqT = nc.dram_tensor("wqT", (D, 256), f16, kind="ExternalInput").ap()
    wkT = nc.dram_tensor("wkT", (D, 256), f16, kind="ExternalInput").ap()
    wvT = nc.dram_tensor("wvT", (D, 256), f16, kind="ExternalInput").ap()
    woT = nc.dram_tensor("woT", (256, D), f16, kind="ExternalInput").ap()
    outT = nc.dram_tensor("outT", (D, S), f32, kind="ExternalOutput").ap()
    if dbg:
        d_qT = nc.dram_tensor("d_qT", (256, S), f16, kind="ExternalOutput").ap()
        d_kT = nc.dram_tensor("d_kT", (256, S), f16, kind="ExternalOutput").ap()
        d_v = nc.dram_tensor("d_v", (S, 258), f16, kind="ExternalOutput").ap()
        d_oT = nc.dram_tensor("d_oT", (256, S), f16, kind="ExternalOutput").ap()
        d_u = nc.dram_tensor("d_u", (S, 2, 258), f32, kind="ExternalOutput").ap()
        d_e = nc.dram_tensor("d_e", (2, 128, 512), f16, kind="ExternalOutput").ap()

    with tile.TileContext(nc) as tc:
        with (
            tc.tile_pool(name="const", bufs=1) as cpool,
            tc.tile_pool(name="persist", bufs=1) as pp,
        ):
            ident = cpool.tile([128, 128], f16, tag="ident")
            make_identity(nc, ident)
            # mask[p, f] = 1 if p <= f else 0 (keys on partitions, q on free)
            maskt = cpool.tile([128, 128], f16, tag="maskt")
            nc.gpsimd.memset(maskt, 1.0)
            ln1ml_ap = cpool.tile([128, 1], f32, tag="ln1ml")
            nc.vector.memset(ln1ml_ap, math.log(1.0 - LAMBDA_INIT))
            nc.gpsimd.affine_select(
                out=maskt, in_=maskt, compare_op=OP.is_ge, fill=0.0,
                base=0, pattern=[[1, 128]], channel_multiplier=-1,
            )

            xT_sb = pp.tile([128, KT, S], f16, tag="xT_sb")
            wq_sb = pp.tile([128, KT, 256], f16, tag="wq_sb")
            wk_sb = pp.tile([128, KT, 256], f16, tag="wk_sb")
            wv_sb = pp.tile([128, KT, 256], f16, tag="wv_sb")
            wo_sb = pp.tile([128, 2, D], f16, tag="wo_sb")
            qT_sb = pp.tile([128, 2, S], f16, tag="qT_sb")
            kT_sb = pp.tile([128, 2, S], f16, tag="kT_sb")
            # v with a ones column appended per head: [v_h0 | 1 | v_h1 | 1]
            v_sb = pp.tile([128, ST, 258], f16, tag="v_sb")
            oT_sb = pp.tile([128, 2, S], f16, tag="oT_sb")

            xT_r = xT.rearrange("(kt p) s -> p kt s", p=128)
            wq_r = wqT.rearrange("(kt p) m -> p kt m", p=128)
            wk_r = wkT.rearrange("(kt p) m -> p kt m", p=128)
            wv_r = wvT.rearrange("(kt p) m -> p kt m", p=128)
            wo_r = woT.rearrange("(kt p) n -> p kt n", p=128)
            for kt in range(KT):
                nc.sync.dma_start(xT_sb[:, kt, :], xT_r[:, kt, :])
                nc.sync.dma_start(wq_sb[:, kt, :], wq_r[:, kt, :])
                nc.sync.dma_start(wk_sb[:, kt, :], wk_r[:, kt, :])
                nc.sync.dma_start(wv_sb[:, kt, :], wv_r[:, kt, :])
            for kt in range(2):
                nc.sync.dma_start(wo_sb[:, kt, :], wo_r[:, kt, :])

            # ---------------- projections ----------------
            with (
                tc.tile_pool(name="ps_qk", bufs=3, space="PSUM") as ps_qk,
                tc.tile_pool(name="ps_v", bufs=2, space="PSUM") as ps_v,
            ):
                def proj_qk(w_sb, dst_sb, mt, evac_engine):
                    for half in range(2):
                        ps = ps_qk.tile([128, 1024], f32, tag="qk")
                        for kt in range(KT):
                            for ns in range(2):
                                nc.tensor.matmul(
                                    ps[:, ns * 512:(ns + 1) * 512],
                                    lhsT=w_sb[:, kt, mt * 128:(mt + 1) * 128],
                                    rhs=xT_sb[:, kt,
                                              half * 1024 + ns * 512:
                                              half * 1024 + (ns + 1) * 512],
                                    start=(kt == 0), stop=(kt == KT - 1),
                                )
                        dst = dst_sb[:, mt, half * 1024:(half + 1) * 1024]
                        if evac_engine == "act":
                            nc.scalar.copy(dst, ps[:])
                        else:
                            nc.vector.tensor_copy(dst, ps[:])

                # head-0 q/k first so attention can start early
                proj_qk(wq_sb, qT_sb, 0, "act")
                proj_qk(wk_sb, kT_sb, 0, "vector")
                proj_qk(wq_sb, qT_sb, 1, "act")
                proj_qk(wk_sb, kT_sb, 1, "vector")

                for st in range(ST):
                    ps = ps_v.tile([128, 256], f32, tag="v")
                    for kt in range(KT):
                        nc.tensor.matmul(
                            ps[:],
                            lhsT=xT_sb[:, kt, st * 128:(st + 1) * 128],
                            rhs=wv_sb[:, kt, :],
                            start=(kt == 0), stop=(kt == KT - 1),
                        )
                    eng = nc.vector if st % 2 == 0 else nc.scalar
                    if st % 2 == 0:
                        nc.vector.tensor_copy(v_sb[:, st, 0:128], ps[:, 0:128])
                        nc.vector.tensor_copy(v_sb[:, st, 129:257], ps[:, 128:256])
                    else:
                        nc.scalar.copy(v_sb[:, st, 0:128], ps[:, 0:128])
                        nc.scalar.copy(v_sb[:, st, 129:257], ps[:, 128:256])
                    nc.vector.memset(v_sb[:, st, 128:129], 1.0)
                    nc.vector.memset(v_sb[:, st, 257:258], 1.0)

            if dbg:
                d_qT_r = d_qT.rearrange("(mt p) s -> p mt s", p=128)
                d_kT_r = d_kT.rearrange("(mt p) s -> p mt s", p=128)
                d_v_r = d_v.rearrange("(st p) c -> p st c", p=128)
                for mt in range(2):
                    nc.sync.dma_start(d_qT_r[:, mt, :], qT_sb[:, mt, :])
                    nc.sync.dma_start(d_kT_r[:, mt, :], kT_sb[:, mt, :])
                for st in range(ST):
                    nc.sync.dma_start(d_v_r[:, st, :], v_sb[:, st, :])

            # ---------------- attention ----------------
            with (
                tc.tile_pool(name="exp0", bufs=20) as ep0,
                tc.tile_pool(name="exp1", bufs=20) as ep1,
                tc.tile_pool(name="ps_s", bufs=3, space="PSUM") as ps_s,
                tc.tile_pool(name="ps_u", bufs=2, space="PSUM") as ps_u,
                tc.tile_pool(name="ps_t", bufs=2, space="PSUM") as ps_t,
                tc.tile_pool(name="nrm", bufs=10) as nrm,
                tc.tile_pool(name="osb", bufs=3) as osb,
            ):
                def pv_qtile(h, s, i, e0_tiles, e1_tiles):
                    qt = 4 * s + i
                    up = ps_u.tile([128, 258], f32, tag="u")
                    for kt in range(qt + 1):
                        c = i * 128
                        vh = v_sb[:, kt, 129 * h:129 * h + 129]
                        nc.tensor.matmul(
                            up[:, 0:129],
                            lhsT=e0_tiles[kt][:, c:c + 128],
                            rhs=vh,
                            start=(kt == 0), stop=(kt == qt),
                        )
                        nc.tensor.matmul(
                            up[:, 129:258],
                            lhsT=e1_tiles[kt][:, c:c + 128],
                            rhs=vh,
                            start=False, stop=(kt == qt),
                            skip_group_check=True,
                        )
                    # normalization + diff + rmsnorm, all per-partition
                    inv0 = nrm.tile([128, 1], f32, tag="inv0")
                    nc.vector.reciprocal(inv0, up[:, 128:129])
                    inv1 = nrm.tile([128, 1], f32, tag="inv1")
                    nc.vector.reciprocal(inv1, up[:, 257:258])
                    t1 = nrm.tile([128, 128], f32, tag="t1")
                    nc.vector.tensor_scalar(t1, up[:, 129:257], inv1, lam,
                                            OP.mult, OP.mult)
                    oq = nrm.tile([128, 128], f32, tag="oq")
                    nc.vector.scalar_tensor_tensor(
                        oq, up[:, 0:128], inv0, t1, OP.mult, OP.subtract)
                    sq = nrm.tile([128, 128], f32, tag="sq")
                    ss = nrm.tile([128, 1], f32, tag="ss")
                    nc.vector.scalar_tensor_tensor(
                        sq, oq, 1.0, oq, OP.bypass, OP.mult, accum_out=ss)
                    ms = nrm.tile([128, 1], f32, tag="ms")
                    nc.vector.tensor_scalar(ms, ss, 1.0 / 128.0, EPS,
                                            OP.mult, OP.add)
                    lnm = nrm.tile([128, 1], f32, tag="lnm")
                    nc.scalar.activation(lnm, ms, AF.Ln)
                    r = nrm.tile([128, 1], f32, tag="r")
                    # r = exp(-0.5*ln(ms) + ln(1-lam_init)) = (1-lam_init)*rsqrt(ms)
                    nc.scalar.activation(r, lnm, AF.Exp, bias=ln1ml_ap[:],
                                         scale=-0.5)
                    if dbg:
                        ub = nrm.tile([128, 258], f32, tag="ub")
                        nc.vector.tensor_copy(ub, up[:])
                        nc.sync.dma_start(
                            d_u.rearrange("(qt p) h c -> p qt h c",
                                          p=128)[:, qt, h, :], ub[:])
                    on = osb.tile([128, 128], f16, tag="on")
                    nc.vector.tensor_scalar(on, oq, r, None, OP.mult)
                    pt = ps_t.tile([128, 128], f16, tag="pt")
                    nc.tensor.transpose(pt, on, ident)
                    nc.scalar.copy(oT_sb[:, h, qt * 128:(qt + 1) * 128], pt[:])

                for h in range(2):
                    for s in range(NSTRIP):
                        e0_tiles = {}
                        e1_tiles = {}
                        for kt in range(4 * (s + 1)):
                            col0 = max(0, (kt - 4 * s) * 128)
                            pa = ps_s.tile([128, 512], f32, tag="sc")
                            pb = ps_s.tile([128, 512], f32, tag="sc")
                            nc.tensor.matmul(
                                pa[:, col0:512],
                                lhsT=kT_sb[0:64, h, kt * 128:(kt + 1) * 128],
                                rhs=qT_sb[0:64, h, s * 512 + col0:(s + 1) * 512],
                                start=True, stop=True, tile_position=(0, 0),
                            )
                            nc.tensor.matmul(
                                pb[:, col0:512],
                                lhsT=kT_sb[64:128, h, kt * 128:(kt + 1) * 128],
                                rhs=qT_sb[64:128, h, s * 512 + col0:(s + 1) * 512],
                                start=True, stop=True, tile_position=(64, 0),
                            )
                            e0 = ep0.tile([128, 512], f16, tag="e0")
                            e1 = ep1.tile([128, 512], f16, tag="e1")
                            nc.scalar.activation(e0[:, col0:512], pa[:, col0:512],
                                                 AF.Exp)
                            nc.scalar.activation(e1[:, col0:512], pb[:, col0:512],
                                                 AF.Exp)
                            if kt >= 4 * s:
                                c = col0
                                nc.vector.tensor_mul(e0[:, c:c + 128],
                                                     e0[:, c:c + 128], maskt)
                                nc.vector.tensor_mul(e1[:, c:c + 128],
                                                     e1[:, c:c + 128], maskt)
                            e0_tiles[kt] = e0
                            e1_tiles[kt] = e1
                            if dbg and h == 0 and s == 0 and kt == 0:
                                nc.sync.dma_start(d_e[0], e0[:])
                                nc.sync.dma_start(d_e[1], e1[:])
                            # emit PV for q-tile qt=kt as soon as its exp
                            # column block exists (kt in [4s, 4s+4))
                            if kt >= 4 * s:
                                pv_qtile(h, s, kt - 4 * s, e0_tiles, e1_tiles)

            if dbg:
                d_oT_r = d_oT.rearrange("(mt p) s -> p mt s", p=128)
                for mt in range(2):
                    nc.sync.dma_start(d_oT_r[:, mt, :], oT_sb[:, mt, :])

            # ---------------- out projection ----------------
            with (
                tc.tile_pool(name="ps_o", bufs=4, space="PSUM") as ps_o,
                tc.tile_pool(name="out_sb", bufs=4) as out_pool,
            ):
                outT_r = outT.rearrange("(mt p) s -> p mt s", p=128)
                for mt in range(8):
                    for ns in range(NSTRIP):
                        ps = ps_o.tile([128, 512], f32, tag="o")
                        for kt in range(2):
                            nc.tensor.matmul(
                                ps[:],
                                lhsT=wo_sb[:, kt, mt * 128:(mt + 1) * 128],
                                rhs=oT_sb[:, kt, ns * 512:(ns + 1) * 512],
                                start=(kt == 0), stop=(kt == 1),
                            )
                        ot = out_pool.tile([128, 512], f32, tag="ot")
                        if (mt + ns) % 2 == 0:
                            nc.vector.tensor_copy(ot[:], ps[:])
                        else:
                            nc.scalar.copy(ot[:], ps[:])
                        nc.sync.dma_start(
                            outT_r[:, mt, ns * 512:(ns + 1) * 512], ot[:])

    nc.compile()
    return nc


def _prep_inputs(x, Wq, Wk, Wv, Wo):
    """Build the 8 per-core input maps (host-side shard + transpose)."""
    f16 = np.float16
    xT = [np.ascontiguousarray(x[b].T).astype(f16) for b in range(B)]
    scale = HD ** -0.5
    in_maps = []
    for d in range(N_CORES):
        b, p = divmod(d, 4)
        r0 = 256 * p
        in_maps.append({
            "xT": xT[b],
            "wqT": np.ascontiguousarray(Wq[r0:r0 + 256, :].T * scale).astype(f16),
            "wkT": np.ascontiguousarray(Wk[r0:r0 + 256, :].T).astype(f16),
            "wvT": np.ascontiguousarray(Wv[r0:r0 + 256, :].T).astype(f16),
            "woT": np.ascontiguousarray(Wo[:, r0:r0 + 256].T).astype(f16),
        })
    return in_maps


_CACHED = {}


def _get_program(lam: float):
    # the program depends on inputs only through lam
    key = round(float(lam), 9)
    if key not in _CACHED:
        _CACHED[key] = _build_program(float(lam))
    return _CACHED[key]


def kernel(x, Wq, Wk, Wv, Wo, lq1, lk1, lq2, lk2):
    from concourse.bass_utils import run_bass_kernel_spmd

    x = np.asarray(x, dtype=np.float32)
    Wq = np.asarray(Wq, dtype=np.float32)
    Wk = np.asarray(Wk, dtype=np.float32)
    Wv = np.asarray(Wv, dtype=np.float32)
    Wo = np.asarray(Wo, dtype=np.float32)
    lq1 = np.asarray(lq1, dtype=np.float32)
    lk1 = np.asarray(lk1, dtype=np.float32)
    lq2 = np.asarray(lq2, dtype=np.float32)
    lk2 = np.asarray(lk2, dtype=np.float32)

    lam1 = np.exp(np.sum(lq1 * lk1, dtype=np.float32))
    lam2 = np.exp(np.sum(lq2 * lk2, dtype=np.float32))
    lam = float(lam1 - lam2 + LAMBDA_INIT)

    nc = _get_program(lam)
    in_maps = _prep_inputs(x, Wq, Wk, Wv, Wo)
    res = run_bass_kernel_spmd(nc, in_maps, core_ids=list(range(N_CORES)))

    out = np.empty((B, S, D), dtype=np.float32)
    for b in range(B):
        acc = res.results[4 * b]["outT"].astype(np.float32).copy()
        for p in range(1, 4):
            acc += res.results[4 * b + p]["outT"]
        out[b] = acc.T
    return out


# revision 42
# speedup vs baseline: 1.7886x; 1.7886x over previous
"""DiffAttention Trainium2 kernel.

Full inputs in, full output out. Sharding: 8 cores = (batch b in {0,1}) x
(head-pair p in {0..3}); each core handles one batch element and 2 of the 8
heads (= 4 of the 16 q/k half-heads, 2 v heads, 256 of the 1024 o columns).
Out-projection is column-split: each core produces a full (S, D) partial of
o @ Wo.T restricted to its o columns; host sums the 4 partials per batch.

All device matmuls run in fp16 (1 cycle/row on PE, fp32 PSUM accumulation).
Host pre-transposes operands so every matmul operand is loaded with natural
(contiguous) DMA:
  xT  = x[b].T                  (D, S)   rhs / lhsT for projections
  wqT = (Wq[rows].T) * hd^-0.5  (D, 256) lhsT for q^T projection (scaling folded)
  wkT = Wk[rows].T              (D, 256)
  wvT = Wv[rows].T              (D, 256) rhs for v projection
  woT = Wo[:, cols].T           (256, D) lhsT for out^T projection
Device returns outT_partial (D, S) fp16; host sums 4 partials per batch in
fp32 and transposes back.

Attention math per head h (half-heads e0=2h, e1=2h+1), per q row:
  u_i = exp(s_i) @ v   (unnormalized), sum_i = exp(s_i) @ 1  (fused: rhs=[v|1])
  o   = u0/sum0 - lam * u1/sum1
  o   = o * rsqrt(mean(o^2)+eps) * (1-lam_init);   out = o @ Wo.T
Scores are computed transposed (keys on partitions, q on free dim) so the
exp'd tiles feed the PV matmul directly as the stationary operand. rsqrt is
Newton-Raphson on the DVE (fast-inverse-sqrt seed), batched per (strip, head),
keeping the ACT engine exp-only (single activation table, no reload churn);
the (1-lam_init) factor is folded into the rsqrt argument. The main loop is
strip-major (512 q columns) so each strip's out-projection and output DMA
overlap the next strip's attention. PSUM banks: scores 2x2 (both half-heads
share one 1024-wide tile, exp'd by a single strided ACT op), u 2, and 2
shared by the o^T transposes and the out-projection.
"""

import math

import numpy as np

B = 2
S = 2048
D = 1024
H = 8
HD = 64  # half-head dim
LAMBDA_INIT = 0.8 - 0.6 * math.exp(-0.3 * 6)
EPS = 1e-5

N_CORES = 8
KT = D // 128      # 8 contraction tiles for projections
ST = S // 128      # 16 sequence tiles
NSTRIP = S // 512  # 4 q strips


def _build_program(lam: float, dbg: bool = False):
    import concourse.bass as bass
    import concourse.tile as tile
    from concourse import bacc, mybir
    from concourse.masks import make_identity

    f16 = mybir.dt.float16
    f32 = mybir.dt.float32
    u32 = mybir.dt.uint32
    AF = mybir.ActivationFunctionType
    OP = mybir.AluOpType

    nc = bacc.Bacc("TRN2", target_bir_lowering=False, debug=False,
                   num_devices=N_CORES)

    xT = nc.dram_tensor("xT", (D, S), f16, kind="ExternalInput").ap()
    wqT = nc.dram_tensor("wqT", (D, 256), f16, kind="ExternalInput").ap()
    wkT = nc.dram_tensor("wkT", (D, 256), f16, kind="ExternalInput").ap()
    wvT = nc.dram_tensor("wvT", (D, 256), f16, kind="ExternalInput").ap()
    woT = nc.dram_tensor("woT", (256, D), f16, kind="ExternalInput").ap()
    outT = nc.dram_tensor("outT", (D, S), f16, kind="ExternalOutput").ap()
    if dbg:
        d_qT = nc.dram_tensor("d_qT", (256, S), f16, kind="ExternalOutput").ap()
        d_kT = nc.dram_tensor("d_kT", (256, S), f16, kind="ExternalOutput").ap()
        d_v = nc.dram_tensor("d_v", (S, 258), f16, kind="ExternalOutput").ap()
        d_oT = nc.dram_tensor("d_oT", (256, S), f16, kind="ExternalOutput").ap()
        d_u = nc.dram_tensor("d_u", (S, 2, 258), f32, kind="ExternalOutput").ap()

    with tile.TileContext(nc) as tc:
        with (
            tc.tile_pool(name="const", bufs=1) as cpool,
            tc.tile_pool(name="persist", bufs=1) as pp,
        ):
            ident = cpool.tile([128, 128], f16, tag="ident")
            make_identity(nc, ident)
            # mask[p, f] = 1 if p <= f else 0 (keys on partitions, q on free)
            maskt = cpool.tile([128, 128], f16, tag="maskt")
            nc.gpsimd.memset(maskt, 1.0)
            nc.gpsimd.affine_select(
                out=maskt, in_=maskt, compare_op=OP.is_ge, fill=0.0,
                base=0, pattern=[[1, 128]], channel_multiplier=-1,
            )
            # constants for Newton-Raphson rsqrt (fast-inverse-sqrt seed)
            magic_c = cpool.tile([128, 8], u32, tag="magic_c")
            nc.gpsimd.memset(magic_c, 0x5F3759DF)
            one_u = cpool.tile([128, 8], u32, tag="one_u")
            nc.gpsimd.memset(one_u, 1)

            wo_sb = pp.tile([128, 2, D], f16, tag="wo_sb")
            qT_sb = pp.tile([128, 2, S], f16, tag="qT_sb")
            kT_sb = pp.tile([128, 2, S], f16, tag="kT_sb")
            # v with a ones column appended per head: [v_h0 | 1 | v_h1 | 1]
            v_sb = pp.tile([128, ST, 258], f16, tag="v_sb")
            oT_sb = pp.tile([128, 2, S], f16, tag="oT_sb")

            nc.gpsimd.dma_start(
                wo_sb[:, :, :],
                woT.rearrange("(kt p) n -> p kt n", p=128)[:, :, :])

            # ---------------- projections ----------------
            from contextlib import ExitStack
            pin_ctx = ExitStack()
            pin = pin_ctx.enter_context(tc.tile_pool(name="proj_in", bufs=1))
            with (
                tc.tile_pool(name="ps_qk", bufs=3, space="PSUM") as ps_qk,
                tc.tile_pool(name="ps_v", bufs=2, space="PSUM") as ps_v,
            ):
                xT_sb = pin.tile([128, KT, S], f16, tag="xT_sb")
                wq_sb = pin.tile([128, KT, 256], f16, tag="wq_sb")
                wk_sb = pin.tile([128, KT, 256], f16, tag="wk_sb")
                wv_sb = pin.tile([128, KT, 256], f16, tag="wv_sb")
                xT_r = xT.rearrange("(kt p) s -> p kt s", p=128)
                wq_r = wqT.rearrange("(kt p) m -> p kt m", p=128)
                wk_r = wkT.rearrange("(kt p) m -> p kt m", p=128)
                wv_r = wvT.rearrange("(kt p) m -> p kt m", p=128)
                nc.scalar.dma_start(wq_sb[:, :, :], wq_r[:, :, :])
                nc.scalar.dma_start(wk_sb[:, :, :], wk_r[:, :, :])
                nc.gpsimd.dma_start(wv_sb[:, :, :], wv_r[:, :, :])
                for kt2 in range(4):
                    nc.sync.dma_start(xT_sb[:, 2 * kt2:2 * kt2 + 2, :],
                                      xT_r[:, 2 * kt2:2 * kt2 + 2, :])

                def proj_qk(w_sb, dst_sb, mt, evac_engine):
                    for half in range(2):
                        ps = ps_qk.tile([128, 1024], f32, tag="qk")
                        for kt in range(KT):
                            for ns in range(2):
                                nc.tensor.matmul(
                                    ps[:, ns * 512:(ns + 1) * 512],
                                    lhsT=w_sb[:, kt, mt * 128:(mt + 1) * 128],
                                    rhs=xT_sb[:, kt,
                                              half * 1024 + ns * 512:
                                              half * 1024 + (ns + 1) * 512],
                                    start=(kt == 0), stop=(kt == KT - 1),
                                )
                        dst = dst_sb[:, mt, half * 1024:(half + 1) * 1024]
                        if evac_engine == "act":
                            nc.scalar.copy(dst, ps[:])
                        else:
                            nc.vector.tensor_copy(dst, ps[:])

                # v first (PV needs all of it), then head-0 q/k so
                # head-0 attention starts early; head-1 q/k are emitted
                # lazily inside the first strip (using ps_o slots).
                for st in range(ST):
                    ps = ps_v.tile([128, 256], f32, tag="v")
                    for kt in range(KT):
                        nc.tensor.matmul(
                            ps[:],
                            lhsT=xT_sb[:, kt, st * 128:(st + 1) * 128],
                            rhs=wv_sb[:, kt, :],
                            start=(kt == 0), stop=(kt == KT - 1),
                        )
                    nc.vector.tensor_copy(v_sb[:, st, 0:128], ps[:, 0:128])
                    nc.vector.tensor_copy(v_sb[:, st, 129:257], ps[:, 128:256])
                    nc.vector.memset(v_sb[:, st, 128:129], 1.0)
                    nc.vector.memset(v_sb[:, st, 257:258], 1.0)

                proj_qk(wq_sb, qT_sb, 0, "vector")
                proj_qk(wk_sb, kT_sb, 0, "vector")

            if dbg:
                d_qT_r = d_qT.rearrange("(mt p) s -> p mt s", p=128)
                d_kT_r = d_kT.rearrange("(mt p) s -> p mt s", p=128)
                d_v_r = d_v.rearrange("(st p) c -> p st c", p=128)
                for mt in range(2):
                    nc.sync.dma_start(d_qT_r[:, mt, :], qT_sb[:, mt, :])
                    nc.sync.dma_start(d_kT_r[:, mt, :], kT_sb[:, mt, :])
                for st in range(ST):
                    nc.sync.dma_start(d_v_r[:, st, :], v_sb[:, st, :])

            pin_ctx.close()

            # ---------------- attention + per-strip out projection ----------
            with (
                tc.tile_pool(name="e0p", bufs=20) as e0pool,
                tc.tile_pool(name="e1p", bufs=20) as e1pool,
                tc.tile_pool(name="ps_s", bufs=2, space="PSUM") as ps_s,
                tc.tile_pool(name="ps_u", bufs=2, space="PSUM") as ps_u,
                tc.tile_pool(name="ps_o", bufs=2, space="PSUM") as ps_o,
                tc.tile_pool(name="nrm", bufs=8) as nrm,
                tc.tile_pool(name="nrm_big", bufs=3) as nrm_big,
                tc.tile_pool(name="osb", bufs=3) as osb,
                tc.tile_pool(name="out_sb", bufs=2) as out_pool,
            ):
                epools = {0: e0pool, 1: e1pool}
                outT_r = outT.rearrange("(mt p) s -> p mt s", p=128)

                def pv_qtile(h, s, i, e0_tiles, e1_tiles, oq_s, ss_s):
                    qt = 4 * s + i
                    up = ps_u.tile([128, 258], f32, tag="u")
                    for kt in range(qt + 1):
                        c = i * 128
                        vh = v_sb[:, kt, 129 * h:129 * h + 129]
                        nc.tensor.matmul(
                            up[:, 0:129],
                            lhsT=e0_tiles[kt][:, c:c + 128],
                            rhs=vh,
                            start=(kt == 0), stop=(kt == qt),
                        )
                        nc.tensor.matmul(
                            up[:, 129:258],
                            lhsT=e1_tiles[kt][:, 512 + c:512 + c + 128],
                            rhs=vh,
                            start=False, stop=(kt == qt),
                            skip_group_check=True,
                        )
                    # normalized diff: oq = u0/s0 - lam*u1/s1 (per-partition)
                    inv0 = nrm.tile([128, 1], f32, tag="inv0")
                    nc.vector.reciprocal(inv0, up[:, 128:129])
                    inv1 = nrm.tile([128, 1], f32, tag="inv1")
                    nc.vector.reciprocal(inv1, up[:, 257:258])
                    t1 = nrm.tile([128, 128], f32, tag="t1")
                    nc.vector.tensor_scalar(t1, up[:, 129:257], inv1, lam,
                                            OP.mult, OP.mult)
                    oq = oq_s[:, i, :]
                    nc.vector.scalar_tensor_tensor(
                        oq, up[:, 0:128], inv0, t1, OP.mult, OP.subtract)
                    sq = nrm.tile([128, 128], f32, tag="sq")
                    nc.vector.scalar_tensor_tensor(
                        sq, oq, 1.0, oq, OP.bypass, OP.mult,
                        accum_out=ss_s[:, i:i + 1])
                    if dbg:
                        ub = nrm.tile([128, 258], f32, tag="ub")
                        nc.vector.tensor_copy(ub, up[:])
                        nc.sync.dma_start(
                            d_u.rearrange("(qt p) h c -> p qt h c",
                                          p=128)[:, qt, h, :], ub[:])

                def norm_tail(h, s, oq_s, ss_s):
                    """Newton rsqrt over the strip's 4 q-tiles, then scale,
                    transpose and evacuate o^T."""
                    ms = nrm.tile([128, 4], f32, tag="ms")
                    il2 = 1.0 / ((1.0 - LAMBDA_INIT) ** 2)
                    nc.vector.tensor_scalar(ms, ss_s, il2 / 128.0, EPS * il2,
                                            OP.mult, OP.add)
                    y0 = nrm.tile([128, 4], u32, tag="y0")
                    nc.vector.tensor_tensor(y0, ms.bitcast(u32),
                                            one_u[:, 0:4],
                                            OP.logical_shift_right)
                    nc.vector.tensor_tensor(y0, magic_c[:, 0:4], y0,
                                            OP.subtract)
                    yf = y0.bitcast(f32)
                    t2 = nrm.tile([128, 4], f32, tag="t2")
                    r_all = nrm.tile([128, 4], f32, tag="r_all")
                    nc.vector.tensor_mul(t2, yf, yf)
                    nc.vector.tensor_mul(t2, t2, ms)
                    nc.vector.tensor_scalar(t2, t2, -0.5, 1.5, OP.mult, OP.add)
                    nc.vector.tensor_mul(r_all, yf, t2)
                    nc.vector.tensor_mul(t2, r_all, r_all)
                    nc.vector.tensor_mul(t2, t2, ms)
                    nc.vector.tensor_scalar(t2, t2, -0.5, 1.5, OP.mult, OP.add)
                    nc.vector.tensor_mul(r_all, r_all, t2)
                    for i in range(4):
                        qt = 4 * s + i
                        on = osb.tile([128, 128], f16, tag="on")
                        nc.vector.tensor_scalar(on, oq_s[:, i, :],
                                                r_all[:, i:i + 1], None,
                                                OP.mult)
                        pt = ps_o.tile([128, 128], f16, tag="o")
                        nc.tensor.transpose(pt, on, ident)
                        nc.vector.tensor_copy(
                            oT_sb[:, h, qt * 128:(qt + 1) * 128], pt[:])

                def emit_qk_mt1():
                    for w_sb_, dst_sb_ in ((wq_sb, qT_sb), (wk_sb, kT_sb)):
                        for ns4 in range(4):
                            ps = ps_o.tile([128, 512], f32, tag="o")
                            for kt in range(KT):
                                nc.tensor.matmul(
                                    ps[:],
                                    lhsT=w_sb_[:, kt, 128:256],
                                    rhs=xT_sb[:, kt,
                                              ns4 * 512:(ns4 + 1) * 512],
                                    start=(kt == 0), stop=(kt == KT - 1),
                                )
                            nc.vector.tensor_copy(
                                dst_sb_[:, 1, ns4 * 512:(ns4 + 1) * 512],
                                ps[:])

                for si, s in enumerate([3, 2, 1, 0]):
                    for h in range(2):
                        if si == 0 and h == 1:
                            emit_qk_mt1()
                        e0_tiles = {}
                        e1_tiles = {}
                        oq_s = nrm_big.tile([128, 4, 128], f32, tag="oq_s")
                        ss_s = nrm_big.tile([128, 4], f32, tag="ss_s")
                        for kt in range(4 * (s + 1)):
                            col0 = max(0, (kt - 4 * s) * 128)
                            pa = ps_s.tile([128, 1024], f32, tag="sc")
                            nc.tensor.matmul(
                                pa[:, col0:512],
                                lhsT=kT_sb[0:64, h, kt * 128:(kt + 1) * 128],
                                rhs=qT_sb[0:64, h, s * 512 + col0:(s + 1) * 512],
                                start=True, stop=True, tile_position=(0, 0),
                            )
                            nc.tensor.matmul(
                                pa[:, 512 + col0:1024],
                                lhsT=kT_sb[64:128, h, kt * 128:(kt + 1) * 128],
                                rhs=qT_sb[64:128, h, s * 512 + col0:(s + 1) * 512],
                                start=True, stop=True, tile_position=(64, 0),
                                skip_group_check=True,
                            )
                            ee = epools[h].tile([128, 1024], f16, tag="e")
                            # exp both half-heads in one ACT op (strided AP
                            # skips the invalid leading columns of each half)
                            w_ = 512 - col0
                            nc.scalar.activation(
                                ee.rearrange("p (b c) -> p b c", b=2)[:, :, col0:512],
                                pa.rearrange("p (b c) -> p b c", b=2)[:, :, col0:512],
                                AF.Exp)
                            if kt >= 4 * s:
                                c = col0
                                nc.gpsimd.tensor_mul(ee[:, c:c + 128],
                                                     ee[:, c:c + 128], maskt)
                                nc.gpsimd.tensor_mul(ee[:, 512 + c:512 + c + 128],
                                                     ee[:, 512 + c:512 + c + 128],
                                                     maskt)
                            e0_tiles[kt] = ee
                            e1_tiles[kt] = ee
                            if kt >= 4 * s:
                                pv_qtile(h, s, kt - 4 * s, e0_tiles, e1_tiles,
                                         oq_s, ss_s)
                        norm_tail(h, s, oq_s, ss_s)

                    # out projection for this strip (needs both heads' oT)
                    ot = out_pool.tile([128, 8, 512], f16, tag="ot")
                    for mt in range(8):
                        ps = ps_o.tile([128, 512], f32, tag="o")
                        for kt in range(2):
                            nc.tensor.matmul(
                                ps[:],
                                lhsT=wo_sb[:, kt, mt * 128:(mt + 1) * 128],
                                rhs=oT_sb[:, kt, s * 512:(s + 1) * 512],
                                start=(kt == 0), stop=(kt == 1),
                            )
                        nc.vector.tensor_copy(ot[:, mt, :], ps[:])
                    nc.sync.dma_start(outT_r[:, :, s * 512:(s + 1) * 512], ot[:])

            if dbg:
                d_oT_r = d_oT.rearrange("(mt p) s -> p mt s", p=128)
                for mt in range(2):
                    nc.sync.dma_start(d_oT_r[:, mt, :], oT_sb[:, mt, :])

    nc.compile()
    return nc


def _prep_inputs(x, Wq, Wk, Wv, Wo):
    """Build the 8 per-core input maps (host-side shard + transpose)."""
    f16 = np.float16
    xT = [np.ascontiguousarray(x[b].T).astype(f16) for b in range(B)]
    scale = HD ** -0.5
    in_maps = []
    for d in range(N_CORES):
        b, p = divmod(d, 4)
        r0 = 256 * p
        in_maps.append({
            "xT": xT[b],
            "wqT": np.ascontiguousarray(Wq[r0:r0 + 256, :].T * scale).astype(f16),
            "wkT": np.ascontiguousarray(Wk[r0:r0 + 256, :].T).astype(f16),
            "wvT": np.ascontiguousarray(Wv[r0:r0 + 256, :].T).astype(f16),
            "woT": np.ascontiguousarray(Wo[:, r0:r0 + 256].T).astype(f16),
        })
    return in_maps


_CACHED = {}


def _get_program(lam: float):
    # the program depends on inputs only through lam
    key = round(float(lam), 9)
    if key not in _CACHED:
        _CACHED[key] = _build_program(float(lam))
    return _CACHED[key]


def kernel(x, Wq, Wk, Wv, Wo, lq1, lk1, lq2, lk2):
    from concourse.bass_utils import run_bass_kernel_spmd

    x = np.asarray(x, dtype=np.float32)
    Wq = np.asarray(Wq, dtype=np.float32)
    Wk = np.asarray(Wk, dtype=np.float32)
    Wv = np.asarray(Wv, dtype=np.float32)
    Wo = np.asarray(Wo, dtype=np.float32)
    lq1 = np.asarray(lq1, dtype=np.float32)
    lk1 = np.asarray(lk1, dtype=np.float32)
    lq2 = np.asarray(lq2, dtype=np.float32)
    lk2 = np.asarray(lk2, dtype=np.float32)

    lam1 = np.exp(np.sum(lq1 * lk1, dtype=np.float32))
    lam2 = np.exp(np.sum(lq2 * lk2, dtype=np.float32))
    lam = float(lam1 - lam2 + LAMBDA_INIT)

    nc = _get_program(lam)
    in_maps = _prep_inputs(x, Wq, Wk, Wv, Wo)
    res = run_bass_kernel_spmd(nc, in_maps, core_ids=list(range(N_CORES)))

    out = np.empty((B, S, D), dtype=np.float32)
    for b in range(B):
        acc = res.results[4 * b]["outT"].astype(np.float32)
        for p in range(1, 4):
            acc += res.results[4 * b + p]["outT"].astype(np.float32)
        out[b] = acc.T
    return out
